# revision 20
# baseline (speedup 1.0000x reference)
"""MoE (8 experts, top-5 Boltzmann gate) Trainium2 kernel — routed expert-parallel.

The reference masks 3 of 8 expert outputs to zero per token, so only 5/8 of
the dense FFN FLOPs contribute. The host computes the (tiny) gate, routes
tokens, and each of the 8 NeuronCores runs ONE expert's FFN over only the
~2600 tokens that selected it. Expert weights stay resident in SBUF; tokens
stream through in 512-wide chunks with both matmuls fused (hT in SBUF).
Gate weighting is applied per-partition at PSUM evict; the host scatter-adds
the 8 partial outputs and adds the w @ b2 term.
"""

import numpy as np

# Problem dims (hardcoded per contract)
D_FULL, H_FULL, O_FULL, NEXP = 1024, 4096, 1024, 8
B_FULL = 4096
NCORES = 8
TEMP = float(np.e)
NA = 5  # active experts per token
N_WARMUP_MM = 6  # dependency-free matmuls to bridge input-DMA latency


def build_moe_routed(C, D=D_FULL, H=H_FULL, O=O_FULL, num_devices=NCORES):
    """Per-core Bass/Tile program: one expert, C routed tokens (C % 128 == 0)."""
    from contextlib import ExitStack

    import concourse.bass as bass
    import concourse.tile as tile
    from concourse import bacc, mybir

    f32 = mybir.dt.float32
    f16 = mybir.dt.float16
    P = 128
    assert C % P == 0
    KD, MH, KH, KH2 = D // P, H // P, H // P, H // (2 * P)
    NO = O // 512
    MT = C // P
    chunks = [(i * 512, min(512, C - i * 512)) for i in range((C + 511) // 512)]

    nc = bacc.Bacc(
        "TRN2", target_bir_lowering=False, debug=False, num_devices=num_devices
    )

    # DRAM I/O (host-packed layouts; all per-partition contiguous)
    xt_d = nc.dram_tensor("xt", [len(chunks), P, KD, 512], f16, kind="ExternalInput").ap()
    w1_d = nc.dram_tensor("w1t", [MH, P, KD, P], f16, kind="ExternalInput").ap()
    w2_d = nc.dram_tensor("w2t", [KH2, P, 2, O], f16, kind="ExternalInput").ap()
    b1_d = nc.dram_tensor("b1p", [P, MH], f32, kind="ExternalInput").ap()
    wv_d = nc.dram_tensor("wv", [P, MT], f32, kind="ExternalInput").ap()
    out_d = nc.dram_tensor("out", [C, O], f16, kind="ExternalOutput").ap()

    Relu = mybir.ActivationFunctionType.Relu

    with tile.TileContext(nc) as tc, ExitStack() as ctx:
        const = ctx.enter_context(tc.tile_pool(name="const", bufs=1))
        xtp = ctx.enter_context(tc.tile_pool(name="xt", bufs=3))
        w1p = ctx.enter_context(tc.tile_pool(name="w1", bufs=1))
        w2p = ctx.enter_context(tc.tile_pool(name="w2", bufs=1))
        htp = ctx.enter_context(tc.tile_pool(name="ht", bufs=MH + 1))
        outp = ctx.enter_context(tc.tile_pool(name="outp", bufs=4))
        ps_1 = ctx.enter_context(tc.tile_pool(name="ps_1", bufs=4, space="PSUM"))
        ps_2 = ctx.enter_context(tc.tile_pool(name="ps_2", bufs=4, space="PSUM"))

        # ---- PE warmup: dependency-free matmuls bridge the input-DMA latency
        # and lift the clock ramp before real work arrives.
        wu = const.tile([P, 512], f16, tag="warmup")
        nc.vector.memset(wu[:], 0.0)
        for i in range(N_WARMUP_MM):
            pw = ps_2.tile([P, 512], f32, tag="ps2", name=f"ps_wu{i}")
            nc.tensor.matmul(pw[:], wu[:, 0:P], wu[:], start=True, stop=True)

        # ---- input DMAs. Three HWDGE rings: sync ring carries the
        # mm1-critical xt + w1 stream; gpsimd ring (no compute on that queue)
        # carries consts + w2 + output stores so DMA triggers never block the
        # ACT (relu-evict) or DVE (weight-evict) queues; scalar ring is pure
        # ACTIVATE.
        xtiles = {}

        def dma_chunk(ci, split=False):
            t = xtp.tile([P, KD, 512], f16, tag="xt", name=f"xt{ci}")
            if split:  # per-k slices so mm1 can start on k0 ASAP
                for k in range(KD):
                    nc.sync.dma_start(t[:, k, :], xt_d[ci][:, k, :])
            else:
                nc.sync.dma_start(t[:], xt_d[ci])
            xtiles[ci] = t

        # w1 stream is the chunk-0 critical path: split it over two rings
        # (even m on sync, odd m on gpsimd ahead of w2) and interleave
        # chunk-0's xt k-slices so the first psum groups are fed at DMA pace.
        w1t = [
            w1p.tile([P, KD, P], f16, tag=f"w1_{m}", name=f"w1_{m}")
            for m in range(MH)
        ]
        for m in range(1, MH, 2):
            nc.gpsimd.dma_start(w1t[m][:], w1_d[m])
        # the first even tiles ride the (still idle) ACT ring, ahead of the
        # first ACTIVATE which isn't needed until the first psum group is done
        for m in (2, 4, 6):
            nc.scalar.dma_start(w1t[m][:], w1_d[m])
        t0 = xtp.tile([P, KD, 512], f16, tag="xt", name="xt0")
        nc.sync.dma_start(t0[:, 0, :], xt_d[0][:, 0, :])
        nc.sync.dma_start(w1t[0][:], w1_d[0])
        for k in range(1, KD):
            nc.sync.dma_start(t0[:, k, :], xt_d[0][:, k, :])
        xtiles[0] = t0
        for m in range(8, MH, 2):
            nc.sync.dma_start(w1t[m][:], w1_d[m])
        if len(chunks) > 1:
            dma_chunk(1)
        b1_sb = const.tile([P, MH], f32)
        nc.gpsimd.dma_start(b1_sb[:], b1_d[:])
        wv_sb = const.tile([P, MT], f32)
        nc.gpsimd.dma_start(wv_sb[:], wv_d[:])
        w2t = []
        for s in range(KH2):
            t = w2p.tile([P, 2, O], f16, tag=f"w2_{s}")
            nc.gpsimd.dma_start(t[:], w2_d[s])
            w2t.append(t)

        # ---- token-chunk loop: mm1 (all H tiles) then mm2, weights resident.
        for ci, (lo, sz) in enumerate(chunks):
            if ci + 2 < len(chunks):
                dma_chunk(ci + 2)
            xtc = xtiles.pop(ci)
            ht = []
            for m in range(MH):
                ps1 = ps_1.tile([P, 512], f32, tag="ps1", name=f"ps1_{ci}_{m}")
                for k in range(KD):
                    nc.tensor.matmul(
                        ps1[:, 0:sz],
                        w1t[m][:, k, :],
                        xtc[:, k, 0:sz],
                        start=(k == 0),
                        stop=(k == KD - 1),
                    )
                h = htp.tile([P, 512], f16, tag="ht", name=f"ht_{ci}_{m}")
                nc.scalar.activation(
                    h[:, 0:sz], ps1[:, 0:sz], Relu, bias=b1_sb[:, m : m + 1]
                )
                ht.append(h)
            for mt in range(sz // P):
                gm = lo // P + mt
                for o2 in range(NO):
                    ps2 = ps_2.tile([P, 512], f32, tag="ps2", name=f"ps2_{gm}_{o2}")
                    for kh in range(KH):
                        nc.tensor.matmul(
                            ps2[:],
                            ht[kh][:, mt * P : (mt + 1) * P],
                            w2t[kh // 2][:, kh % 2, o2 * 512 : (o2 + 1) * 512],
                            start=(kh == 0),
                            stop=(kh == KH - 1),
                        )
                    ob = outp.tile([P, 512], f16, tag="ob", name=f"ob_{gm}_{o2}")
                    nc.vector.tensor_scalar_mul(ob[:], ps2[:], wv_sb[:, gm : gm + 1])
                    nc.sync.dma_start(
                        out_d[gm * P : (gm + 1) * P, o2 * 512 : (o2 + 1) * 512], ob[:]
                    )

    nc.compile()
    return nc


def route(x, Wg, bg):
    """Host gate: returns (w [B,N] f32 renormalized top-NA weights, mask [B,N])."""
    lg = x.astype(np.float64) @ Wg.T.astype(np.float64) + bg.astype(np.float64)
    lg /= TEMP
    lg -= lg.max(axis=1, keepdims=True)
    p = np.exp(lg)
    p /= p.sum(axis=1, keepdims=True)
    # keep top-NA: threshold at the (N-NA)-th smallest prob
    part = np.partition(p, NEXP - NA - 1, axis=1)
    thr = part[:, NEXP - NA - 1]
    mask = p > thr[:, None]
    kept = mask.sum(axis=1)
    bad = np.nonzero(kept != NA)[0]
    for b in bad:  # exact ties (measure-zero): replicate top_k tie-breaking
        order = np.argsort(-p[b], kind="stable")
        mask[b] = False
        mask[b, order[:NA]] = True
    w = p * mask
    w = w / (w.sum(axis=1, keepdims=True) + 1e-8)
    return w.astype(np.float32), mask


def pack_core_inputs(x, W1, b1, W2, w, mask):
    """Shard by expert; returns (in_maps, idx_list, counts, C)."""
    P = 128
    N, H, D = W1.shape
    O = W2.shape[1]
    KD, MH, KH2 = D // P, H // P, H // (2 * P)

    idx_list = [np.nonzero(mask[:, n])[0] for n in range(N)]
    counts = np.array([len(i) for i in idx_list])
    C = int(np.ceil(counts.max() / P) * P)
    MT = C // P
    nch = (C + 511) // 512

    in_maps = []
    for n in range(N):
        idx = idx_list[n]
        cnt = len(idx)
        xs = x[idx].astype(np.float16)  # [cnt, D]
        xt = np.zeros((P, KD, nch * 512), np.float16)
        xt[:, :, :cnt] = xs.T.reshape(KD, P, cnt).transpose(1, 0, 2)
        xt = np.ascontiguousarray(
            xt.reshape(P, KD, nch, 512).transpose(2, 0, 1, 3)
        )  # [nch, P, KD, 512]
        w1t = np.ascontiguousarray(
            W1[n].reshape(MH, P, KD, P).transpose(0, 3, 2, 1), np.float16
        )  # [m, p_d, k, q_h]
        w2t = np.ascontiguousarray(
            W2[n].T.reshape(KH2, 2, P, O).transpose(0, 2, 1, 3), np.float16
        )  # [kh2, p_h, c, o]
        b1p = np.ascontiguousarray(b1[n].reshape(MH, P).T, np.float32)
        wvec = np.zeros(MT * P, np.float32)
        wvec[:cnt] = w[idx, n]
        wv = np.ascontiguousarray(wvec.reshape(MT, P).T, np.float32)
        in_maps.append({"xt": xt, "w1t": w1t, "w2t": w2t, "b1p": b1p, "wv": wv})
    return in_maps, idx_list, counts, C


_NC_CACHE = {}


def _get_nc(C):
    if C not in _NC_CACHE:
        _NC_CACHE[C] = build_moe_routed(C)
    return _NC_CACHE[C]


def kernel(x, W1, b1, W2, b2, Wg, bg):
    from concourse.bass_utils import run_bass_kernel_spmd

    x = np.asarray(x, np.float32)
    W1 = np.asarray(W1, np.float32)
    b1 = np.asarray(b1, np.float32)
    W2 = np.asarray(W2, np.float32)
    b2 = np.asarray(b2, np.float32)
    Wg = np.asarray(Wg, np.float32)
    bg = np.asarray(bg, np.float32)

    w, mask = route(x, Wg, bg)
    in_maps, idx_list, counts, C = pack_core_inputs(x, W1, b1, W2, w, mask)
    nc = _get_nc(C)
    try:
        res = run_bass_kernel_spmd(nc, in_maps, core_ids=list(range(NCORES)))
    except Exception:
        # transient NRT exec-unit failures have been observed to clear on retry
        res = run_bass_kernel_spmd(nc, in_maps, core_ids=list(range(NCORES)))

    out = w @ b2  # the (sum_n w_n * b2_n) term, [B, O]
    for n in range(NCORES):
        out[idx_list[n]] += res.results[n]["out"][: counts[n]].astype(np.float32)
    return np.ascontiguousarray(out, np.float32)
